# revision 1
# baseline (speedup 1.0000x reference)
"""Trainium2 Bass kernel for nn_DNM_Conv (LayerNorm -> synapse contraction ->
dendritic weighting -> GELU -> residual multiply).

Algebraic reduction of the reference:
    y = LayerNorm(x)                                  (b, n, d)
    t[b,o,d] = sum_n W[o,n] * y[b,n,d] + c[o]
        where W[o,n] = sum_m dw[o,m]*sw[o,m,n],  c[o] = sum_{m,n} dw[o,m]*sb[o,m,n]
    out = x * (gelu_erf(t) + 1)                       (o == n == 196)

Key structure:
  * LN scale folded into the weights: Wr[o,n] = W[o,n]*rstd[n] (196-wide
    per-batch scale instead of a 768-wide normalize pass over x);
    t = Wr@x - (Wr@mu) + c.  The mu term rides as a 769th rhs column
    (bn_aggr writes [mean,var] directly into x-tile columns 768:770; the
    host pads x rows to 770 so the x loads are fully contiguous).
  * Residual: output is seeded with x by a DRAM->DRAM copy; the kernel
    computes p = x*gelu(t) on DVE (fp16 2x mode) and accumulates it into
    DRAM with a gpsimd accumulate-DMA.
  * ACT table sets: rstd (abs_reciprocal_sqrt set) and Gelu live in
    different sets, so rstd work is batched in two groups (batches 0-3,
    4-7) pinned with explicit deps -> 4 table loads, and the PE/PSUM
    pipeline unblocks after the first group instead of the last batch.

Distribution: data-parallel over batch, 8 batches per core on 8 cores.
Datapath fp16 (host casts), fp32 PSUM accumulation, fp32 LN statistics.
"""

import numpy as np

B, N, D, O, M = 64, 196, 768, 196, 2
N_CORES = 8
BPC = B // N_CORES          # batches per core
NPAIR = BPC // 2            # batch pairs (DMA granularity)
NA, NB = 128, 68            # n partition split
OA, OB = 128, 68            # o partition split
DC = 384                    # matmul moving free-dim chunk
XW = D + 2                  # x row width (768 data + [mean, var] slots)
LN_EPS = 1e-5

_NC_CACHE = {}


def _build_nc(nontrivial_ln):
    import concourse.bacc as bacc
    import concourse.tile as tile
    import concourse.bass as bass
    from concourse.tile import add_dep_helper
    from concourse import mybir
    from contextlib import ExitStack

    F32 = mybir.dt.float32
    F16 = mybir.dt.float16
    AF = mybir.ActivationFunctionType
    OP = mybir.AluOpType

    nc = bacc.Bacc()
    x_d = nc.declare_dram_parameter("x", [BPC, N, XW], F16, isOutput=False)
    wt_d = nc.declare_dram_parameter("wt", [N, O], F16, isOutput=False)
    c_d = nc.declare_dram_parameter("c", [O, 1], F32, isOutput=False)
    if nontrivial_ln:
        lnw_d = nc.declare_dram_parameter("lnw", [1, D], F32, isOutput=False)
        lnbe_d = nc.declare_dram_parameter("lnbe", [O, D], F32, isOutput=False)
    out_d = nc.declare_dram_parameter("out", [BPC, N, D], F16, isOutput=True)

    x_pair = x_d.ap().rearrange("(q j) n d -> q n j d", j=2)    # (4, 196, 2, 770)
    out_pair = out_d.ap().rearrange("(q j) n d -> q n j d", j=2)

    with tile.TileContext(nc) as tc, ExitStack() as ctx:
        const = ctx.enter_context(tc.tile_pool(name="const", bufs=1))
        xpool = ctx.enter_context(tc.tile_pool(name="xpool", bufs=NPAIR))
        stpool = ctx.enter_context(tc.tile_pool(name="stpool", bufs=BPC))
        wrpool = ctx.enter_context(tc.tile_pool(name="wrpool", bufs=4))
        gpool = ctx.enter_context(tc.tile_pool(name="gpool", bufs=3))
        opool = ctx.enter_context(tc.tile_pool(name="opool", bufs=2))
        psum = ctx.enter_context(tc.tile_pool(name="psum", bufs=2, space="PSUM"))

        # ---- constants (sync queue, before x loads; all tiny) ----
        wt_a = const.tile([NA, O], F16, tag="wt_a")
        wt_b = const.tile([NB, O], F16, tag="wt_b")
        nc.sync.dma_start(out=wt_a[:], in_=wt_d[0:NA, :])
        nc.sync.dma_start(out=wt_b[:], in_=wt_d[NA:N, :])
        c_a = const.tile([OA, 1], F32, tag="c_a")
        c_b = const.tile([OB, 1], F32, tag="c_b")
        nc.sync.dma_start(out=c_a[:], in_=c_d[0:OA, :])
        nc.sync.dma_start(out=c_b[:], in_=c_d[OA:O, :])
        eps_t = const.tile([128, 1], F32, tag="eps")
        nc.vector.memset(eps_t[:], LN_EPS)
        if nontrivial_ln:
            lnw_t = const.tile([128, D], F32, tag="lnw")
            lnw_bcast = bass.AP(tensor=lnw_d.ap().tensor, offset=0,
                                ap=[[0, 128], [1, D]])
            nc.sync.dma_start(out=lnw_t[:], in_=lnw_bcast)
            lnbe_a = const.tile([OA, D], F32, tag="lnbe_a")
            lnbe_b = const.tile([OB, D], F32, tag="lnbe_b")
            nc.sync.dma_start(out=lnbe_a[:], in_=lnbe_d[0:OA, :])
            nc.sync.dma_start(out=lnbe_b[:], in_=lnbe_d[OA:O, :])

        nsplit = ((0, NA), (NA, NB))
        osplit = ((0, OA, c_a), (OA, OB, c_b))

        # ---- x loads (sync queue, front; fully contiguous 770-wide rows) ----
        xtiles = []  # [pair][ci] -> (pn, 2, 770) fp16
        for q in range(NPAIR):
            xq = []
            for ci, (p0, pn) in enumerate(nsplit):
                xt = xpool.tile([pn, 2, XW], F16, tag=f"x{ci}")
                nc.sync.dma_start(out=xt[:], in_=x_pair[q, p0:p0 + pn, :, :])
                xq.append(xt)
            xtiles.append(xq)

        # ---- bn stats for all batches (DVE): [mean,var] -> x cols 768:770 ----
        for i in range(BPC):
            q, j = divmod(i, 2)
            for ci, (p0, pn) in enumerate(nsplit):
                xt = xtiles[q][ci]
                stats = stpool.tile([pn, 2, 6], F32, tag=f"stats{ci}")
                xg = xt[:, j, 0:D].rearrange("p (s f) -> p s f", s=2)
                for s in range(2):
                    nc.vector.bn_stats(out=stats[:, s, :], in_=xg[:, s, :])
                nc.vector.bn_aggr(out=xt[:, j, D:D + 2], in_=stats[:])

        # ---- helpers to emit per-batch rstd/Wr and per-pair phase B ----
        wrs = {}

        def emit_rstd_wr(i):
            q, j = divmod(i, 2)
            wri = []
            last = None
            for ci, (p0, pn) in enumerate(nsplit):
                xt = xtiles[q][ci]
                rstd = stpool.tile([pn, 1], F32, tag=f"rstd{ci}",
                                   name=f"rstd{i}_{ci}")
                last = nc.scalar.activation(out=rstd[:],
                                            in_=xt[:, j, D + 1:D + 2],
                                            func=AF.Abs_reciprocal_sqrt,
                                            bias=eps_t[0:pn, :], scale=1.0)
                wr = wrpool.tile([pn, O], F16, tag=f"wr{ci}", name=f"wr{i}_{ci}")
                wt_t = wt_a if ci == 0 else wt_b
                if ci == 0:
                    nc.scalar.activation(out=wr[:], in_=wt_t[:], func=AF.Copy,
                                         scale=rstd[:])
                else:
                    nc.vector.tensor_scalar_mul(out=wr[:], in0=wt_t[:],
                                                scalar1=rstd[:])
                wri.append(wr)
            wrs[i] = wri
            return last

        def emit_pair_phase_b(q, first_gelu_dep):
            xs = xtiles[q]
            seed = nc.sync.dma_start(out=out_pair[q, :, :, :],
                                     in_=x_pair[q, :, :, 0:D])
            out_a = opool.tile([NA, 2, D], F16, tag="out0", name=f"out0_{q}")
            out_b = opool.tile([NB, 2, D], F16, tag="out1", name=f"out1_{q}")
            outs = (out_a, out_b)
            gt = [[gpool.tile([on, 2, DC], F16, tag=f"g{oc}{dc}",
                              name=f"g{q}_{oc}{dc}")
                   for dc in range(2)] for oc, (o0, on, c_t) in enumerate(osplit)]
            first_gelu = None
            last_gelu = None
            for j in range(2):
                i = 2 * q + j
                for oc, (o0, on, c_t) in enumerate(osplit):
                    pm1 = psum.tile([on, DC + 1], F32, tag=f"pm{oc}1")
                    for k, wr in enumerate(wrs[i]):
                        nc.tensor.matmul(pm1[:], wr[:, o0:o0 + on],
                                         xs[k][:, j, DC:D + 1],
                                         start=(k == 0), stop=(k == 1))
                    gbias = stpool.tile([on, 1], F32, tag=f"gb{oc}")
                    nc.vector.tensor_tensor(out=gbias[:], in0=c_t[:],
                                            in1=pm1[:, DC:DC + 1],
                                            op=OP.subtract)
                    pm0 = psum.tile([on, DC], F32, tag=f"pm{oc}0")
                    for k, wr in enumerate(wrs[i]):
                        nc.tensor.matmul(pm0[:], wr[:, o0:o0 + on],
                                         xs[k][:, j, 0:DC],
                                         start=(k == 0), stop=(k == 1))

                    for dc, pm in ((1, pm1), (0, pm0)):
                        ds = slice(dc * DC, (dc + 1) * DC)
                        if nontrivial_ln:
                            lnbe_t = lnbe_a if oc == 0 else lnbe_b
                            nc.vector.tensor_scalar_sub(
                                out=pm[:, 0:DC], in0=pm[:, 0:DC],
                                scalar1=pm1[:, DC:DC + 1])
                            nc.vector.tensor_mul(out=pm[:, 0:DC],
                                                 in0=pm[:, 0:DC],
                                                 in1=lnw_t[0:on, ds])
                            nc.vector.tensor_add(out=pm[:, 0:DC],
                                                 in0=pm[:, 0:DC],
                                                 in1=lnbe_t[:, ds])
                            ins = nc.scalar.activation(
                                out=gt[oc][dc][:, j, :], in_=pm[:, 0:DC],
                                func=AF.Gelu, bias=c_t[:], scale=1.0)
                        else:
                            ins = nc.scalar.activation(
                                out=gt[oc][dc][:, j, :], in_=pm[:, 0:DC],
                                func=AF.Gelu, bias=gbias[:], scale=1.0)
                        if first_gelu is None:
                            first_gelu = ins
                            if first_gelu_dep is not None:
                                add_dep_helper(ins.ins, first_gelu_dep.ins,
                                               sync=True,
                                               reason="sqrt-set before gelu-set")
                        last_gelu = ins

            for oc, (o0, on, c_t) in enumerate(osplit):
                for dc in range(2):
                    ds = slice(dc * DC, (dc + 1) * DC)
                    nc.vector.tensor_mul(out=outs[oc][:, :, ds],
                                         in0=gt[oc][dc][:],
                                         in1=xs[oc][:, :, ds])
            for ci, (p0, pn) in enumerate(nsplit):
                acc = nc.gpsimd.dma_start(out=out_pair[q, p0:p0 + pn, :, :],
                                          in_=outs[ci][:],
                                          accum_op=OP.add)
                add_dep_helper(acc.ins, seed.ins, sync=True,
                               reason="accumulate after residual seed")
            return last_gelu

        # ---- two table-set groups: batches 0-3, then 4-7 ----
        GROUPS = ((0, BPC // 2), (BPC // 2, BPC))
        prev_gelu = None
        for (b0, b1) in GROUPS:
            last_rstd = None
            for i in range(b0, b1):
                ins = emit_rstd_wr(i)
                if prev_gelu is not None and i == b0:
                    add_dep_helper(ins.ins, prev_gelu.ins, sync=True,
                                   reason="second sqrt-set after gelu group")
                last_rstd = ins
            for q in range(b0 // 2, b1 // 2):
                prev_gelu = emit_pair_phase_b(q, last_rstd)
                last_rstd = None

    nc.compile()
    return nc


def kernel(x, ln_w, ln_b, sw, sb, dw, _trace=False):
    from concourse.bass_utils import run_bass_kernel_spmd

    x = np.asarray(x, dtype=np.float32)
    ln_w = np.asarray(ln_w, dtype=np.float32)
    ln_b = np.asarray(ln_b, dtype=np.float32)
    sw = np.asarray(sw, dtype=np.float32)
    sb = np.asarray(sb, dtype=np.float32)
    dw = np.asarray(dw, dtype=np.float32)

    x16 = np.zeros((B, N, XW), dtype=np.float16)
    x16[:, :, 0:D] = x.astype(np.float16)

    # Fold dendritic weights into the synapse contraction (host, ~0.1 ms).
    W = np.einsum("om,omn->on", dw, sw)            # (o, n)
    WT = np.ascontiguousarray(W.T.astype(np.float16))
    c = np.einsum("om,om->o", dw, sb.sum(-1)).astype(np.float32)[:, None]

    nontrivial_ln = not (np.all(ln_w == 1.0) and np.all(ln_b == 0.0))
    key = bool(nontrivial_ln)
    if key not in _NC_CACHE:
        _NC_CACHE[key] = _build_nc(nontrivial_ln)
    nc = _NC_CACHE[key]

    in_maps = []
    for i in range(N_CORES):
        m = {"x": x16[i * BPC:(i + 1) * BPC], "wt": WT, "c": c}
        if nontrivial_ln:
            m["lnw"] = ln_w[None, :]
            m["lnbe"] = (W.sum(-1)[:, None] * ln_b[None, :]).astype(np.float32)
        in_maps.append(m)

    res = run_bass_kernel_spmd(nc, in_maps, core_ids=list(range(N_CORES)),
                               trace=_trace)
    out = np.concatenate([res.results[i]["out"] for i in range(N_CORES)],
                         axis=0).astype(np.float32)
    if _trace:
        return out, res
    return out

